# revision 1
# baseline (speedup 1.0000x reference)
"""MoE (top-2, 8 experts, SwiGLU + shared expert) on 8 TRN2 NeuronCores.

Strategy: expert-parallel. Host computes the (tiny) router + dispatch
indices, gathers each expert's tokens into a padded [C, DIM] block
(pre-scaled by router score), and ships core e:
  - its expert's tokens, feature-major  xrT   [DIM, C]
  - a 1/8 token shard for the shared expert  xsT [DIM, S]
  - its expert weights w13 (w1/w3 column-interleaved) and w2
  - the shared-expert weights (replicated)
Each core runs two dense SwiGLU MLPs entirely feature-major (activations
are the moving operand, weights stationary), so no transposes anywhere.
Host scatter-adds the routed outputs into the shared-expert output.

The device program is RAW Bass (manual semaphores): the walrus build in
this container accepts at most one inline sync wait per instruction, so
Tile's auto-generated multi-wait sync_info cannot compile.  All waits
are standalone wait_ge instructions; every instruction carries at most
one then_inc, extra increments are standalone sem_inc.

Engine roles:
  sync  (SP) : input + weight streaming DMAs (qSPDynamicHW ring, FIFO)
  tensor(PE) : all matmuls
  scalar(ACT): silu eviction from PSUM; output DMAs (qActDynamicHW ring)
  vector(DVE): silu*h3 multiply into g; PSUM->SBUF output copies
"""

from contextlib import ExitStack

import numpy as np

import concourse.bass as bass
import concourse.mybir as mybir

DIM = 1024
HIDDEN = 1024
NUM_EXPERTS = 8
TOP_K = 2
N_CORES = 8
P = 128
KT = DIM // P

# dtype used for the matmul operands on-device.
MM_DT = mybir.dt.float32r

W_RING = 8   # weight-tile buffer ring depth
S_RING = 4   # silu scratch ring
O_RING = 3   # output tile ring
NSEM_W = 12  # weight-DMA completion sem ring (> W_RING: skew-free reuse)
NSEM_OD = 4  # output-DMA completion sem ring (> O_RING)
BANKS_PER_PASS = 4  # PSUM accumulator banks per pass (4 = double-banked)


def _chunks(total, maxc=512):
    if total <= maxc:
        return [(0, total)]
    if total <= 2 * maxc:
        h = ((total + 1) // 2 + 15) // 16 * 16
        return [(0, h), (h, total - h)]
    out, off = [], 0
    while total - off > maxc:
        out.append((off, maxc))
        off += maxc
    out.append((off, total - off))
    return out


class Plan:
    """Per-engine instruction streams with planned semaphore counters."""

    ENGINES = ("sync", "tensor", "scalar", "vector")

    def __init__(self):
        self.streams = {e: [] for e in self.ENGINES}
        self.cnt = {}  # sem name -> planned cumulative increments
        self._waited = {}  # (eng, sem) -> max value already waited

    def wait(self, eng, sem, val):
        val = int(val)
        if val <= 0 or self._waited.get((eng, sem), 0) >= val:
            return
        self._waited[(eng, sem)] = val
        self.streams[eng].append(("wait", sem, val))

    def op(self, eng, fn, incs=()):
        self.streams[eng].append(("op", fn, tuple(incs)))
        for s, v in incs:
            self.cnt[s] = self.cnt.get(s, 0) + v


def plan_mlp(plan, st, T, w13_name, w2_name, rhs_x, g_tiles, out_name):
    """Plan one SwiGLU MLP (phases A+B) into the streams.

    Every instruction carries at most ONE then_inc; all cross-engine
    signaling is completion-accurate (the inc rides on the instruction
    whose completion it reports).  Semaphores:
      w  : +16 per SP DMA completion (inputs + weights, FIFO ring)
      mm : +1 on the last matmul of each (pass,k) burst -> burst done
      s  : +1 per silu (ACT) completion
      g  : +1 per gated-multiply (DVE) completion
      o  : +1 per PSUM->SBUF output-chunk copy (DVE) completion
      od : +16 per output DMA (ACT ring) completion
    """
    nch = _chunks(T)
    ncn = len(nch)
    mg = max(2, BANKS_PER_PASS // ncn) if ncn <= 2 else 2  # m-tiles per pass

    g_base = plan.cnt.get("g", 0)

    def weight_dma(dram_name, k, m0, mcols):
        st["w_idx"] += 1
        widx = st["w_idx"]
        slot = widx % W_RING
        if widx > W_RING:
            plan.wait("sync", "mm", widx - W_RING)
        def fn(e, _slot=slot, _k=k, _m0=m0, _mc=mcols, _nm=dram_name):
            t = st["tens"]
            return e.dma_start(out=t[f"wt{_slot}"][:, :_mc],
                               in_=t[_nm][_k * P:(_k + 1) * P, _m0:_m0 + _mc])
        # dedicated sem ring: sem value is exact per-transfer (the 16
        # per-engine increments of ONE dma), so waits are skew-free.
        wsem = f"w{(widx - 1) % NSEM_W}"
        wval = 16 * ((widx - 1) // NSEM_W + 1)
        plan.op("sync", fn, incs=((wsem, 16),))
        return (wsem, wval), slot, widx

    def bursts(rhs, w_name, m_base, x_load=None):
        """Plan the KT matmul bursts of one pass; returns burst idx of last."""
        for k in range(KT):
            if x_load is not None:
                xsem = x_load(k)      # SP: load x tile k now (single-use sem)
            (wsem, wval), slot, widx = weight_dma(w_name, k, m_base, mg * P)
            if x_load is not None:
                plan.wait("tensor", xsem, 16)
            plan.wait("tensor", wsem, wval)
            if rhs is g_tiles:
                plan.wait("tensor", "g", g_base + ncn * (k + 1))
            n_mc = mg * ncn
            i_mc = 0
            bset = (st["pass_par"] % 2) * 4 if BANKS_PER_PASS == 4 else 0
            for ml in range(mg):
                for ci, (c0, cw) in enumerate(nch):
                    b = bset + ml * ncn + ci
                    if k == 0 and st["bank_rel"][b] is not None:
                        rs, rv = st["bank_rel"][b]
                        plan.wait("tensor", rs, rv)
                    i_mc += 1
                    incs = (("mm", 1),) if i_mc == n_mc else ()
                    def mmop(e, _b=b, _slot=slot, _ml=ml, _k=k, _c0=c0,
                             _cw=cw, _rhs=rhs):
                        t = st["tens"]
                        return e.matmul(
                            t[f"pb{_b}"][:, :_cw],
                            lhsT=t[f"wt{_slot}"][:, _ml * P:(_ml + 1) * P],
                            rhs=_rhs[_k][:, _c0:_c0 + _cw],
                            start=(_k == 0), stop=(_k == KT - 1),
                            skip_group_check=True)
                    plan.op("tensor", mmop, incs=incs)
        return st["w_idx"]

    # ---------------- phase A:  h13 -> g ----------------
    n_pass = (2 * HIDDEN // P) // mg
    for p_i in range(n_pass):
        m0 = p_i * mg * P
        done = bursts(rhs_x, w13_name, m0,
                      x_load=st["x_load"][id(rhs_x)] if p_i == 0 else None)
        bset = (st["pass_par"] % 2) * 4 if BANKS_PER_PASS == 4 else 0
        st["pass_par"] += 1
        for mp in range(mg // 2):
            h = (m0 // P) // 2 + mp
            for ci, (c0, cw) in enumerate(nch):
                b1 = bset + (2 * mp) * ncn + ci
                b3 = bset + (2 * mp + 1) * ncn + ci
                st["s_idx"] += 1
                s_slot = st["s_idx"] % S_RING
                plan.wait("scalar", "mm", done)
                if st["s_rel"][s_slot] is not None:
                    rs, rv = st["s_rel"][s_slot]
                    plan.wait("scalar", rs, rv)
                def silu(e, _s=s_slot, _b=b1, _cw=cw):
                    t = st["tens"]
                    return e.activation(
                        t[f"s{_s}"][:, :_cw], t[f"pb{_b}"][:, :_cw],
                        mybir.ActivationFunctionType.Silu)
                plan.op("scalar", silu, incs=(("s", 1),))
                st["bank_rel"][b1] = ("s", plan.cnt["s"])
                s_need = plan.cnt["s"]
                plan.wait("vector", "mm", done)
                plan.wait("vector", "s", s_need)
                def mul(e, _h=h, _s=s_slot, _b=b3, _c0=c0, _cw=cw):
                    t = st["tens"]
                    return e.tensor_mul(g_tiles[_h][:, _c0:_c0 + _cw],
                                        t[f"s{_s}"][:, :_cw],
                                        t[f"pb{_b}"][:, :_cw])
                plan.op("vector", mul, incs=(("g", 1),))
                st["bank_rel"][b3] = ("g", plan.cnt["g"])
                st["s_rel"][s_slot] = ("g", plan.cnt["g"])

    # ---------------- phase B:  outT = w2.T @ g ----------------
    n_pass = (DIM // P) // mg
    for p_i in range(n_pass):
        m0 = p_i * mg * P
        done = bursts(g_tiles, w2_name, m0)
        bset = (st["pass_par"] % 2) * 4 if BANKS_PER_PASS == 4 else 0
        st["pass_par"] += 1
        for ml in range(mg):
            mg_glob = m0 // P + ml
            st["o_idx"] += 1
            o_slot = st["o_idx"] % O_RING
            plan.wait("vector", "mm", done)
            if st["o_rel"][o_slot] is not None:
                rs, rv = st["o_rel"][o_slot]
                plan.wait("vector", rs, rv)
            for ci, (c0, cw) in enumerate(nch):
                b = bset + ml * ncn + ci
                def cp(e, _o=o_slot, _b=b, _c0=c0, _cw=cw):
                    t = st["tens"]
                    return e.tensor_copy(t[f"ot{_o}"][:, _c0:_c0 + _cw],
                                         t[f"pb{_b}"][:, :_cw])
                plan.op("vector", cp, incs=(("o", 1),))
                st["bank_rel"][b] = ("o", plan.cnt["o"])
            o_need = plan.cnt["o"]
            plan.wait("scalar", "o", o_need)
            odsem = f"od{st["od_idx"] % NSEM_OD}"
            odval = 16 * (st["od_idx"] // NSEM_OD + 1)
            st["od_idx"] += 1
            st["o_rel"][o_slot] = (odsem, odval)
            def odma(e, _o=o_slot, _m=mg_glob, _T=T, _nm=out_name):
                t = st["tens"]
                return e.dma_start(out=t[_nm][_m * P:(_m + 1) * P, :],
                                   in_=t[f"ot{_o}"][:, :_T])
            plan.op("scalar", odma, incs=((odsem, 16),))


def build_program(C, S, mm_dt=MM_DT):
    nc = bass.Bass()
    tens = {}
    tens["xrT"] = nc.declare_dram_parameter("xrT", [DIM, C], mm_dt, isOutput=False)
    tens["xsT"] = nc.declare_dram_parameter("xsT", [DIM, S], mm_dt, isOutput=False)
    tens["w13"] = nc.declare_dram_parameter("w13", [DIM, 2 * HIDDEN], mm_dt,
                                            isOutput=False)
    tens["w2"] = nc.declare_dram_parameter("w2", [HIDDEN, DIM], mm_dt,
                                           isOutput=False)
    tens["w13s"] = nc.declare_dram_parameter("w13s", [DIM, 2 * HIDDEN], mm_dt,
                                             isOutput=False)
    tens["w2s"] = nc.declare_dram_parameter("w2s", [HIDDEN, DIM], mm_dt,
                                            isOutput=False)
    tens["yrT"] = nc.declare_dram_parameter("yrT", [DIM, C], mybir.dt.float32,
                                            isOutput=True)
    tens["ysT"] = nc.declare_dram_parameter("ysT", [DIM, S], mybir.dt.float32,
                                            isOutput=True)

    cmax = max(_chunks(C), key=lambda c: c[1])[1]
    cmax = max(cmax, S)

    st = {
        "tens": tens, "w_idx": 0, "s_idx": 0, "o_idx": 0, "pass_par": 0,
        "od_idx": 0, "bank_rel": [None] * 8, "s_rel": [None] * S_RING,
        "o_rel": [None] * O_RING, "x_load": {},
    }
    plan = Plan()

    with ExitStack() as ctx:
        # SBUF tensors
        def sb(name, shape, dt):
            tens[name] = ctx.enter_context(nc.sbuf_tensor(name, shape, dt))
        for k in range(KT):
            sb(f"xr{k}", [P, C], mm_dt)
            sb(f"xs{k}", [P, S], mm_dt)
            sb(f"gr{k}", [P, C], mm_dt)
            sb(f"gs{k}", [P, S], mm_dt)
        for r in range(W_RING):
            sb(f"wt{r}", [P, 1024], mm_dt)
        for r in range(S_RING):
            sb(f"s{r}", [P, cmax], mybir.dt.float32)
        for r in range(O_RING):
            sb(f"ot{r}", [P, max(C, S)], mybir.dt.float32)
        for b in range(8):
            tens[f"pb{b}"] = ctx.enter_context(
                nc.psum_tensor(f"pb{b}", [P, 512], mybir.dt.float32))

        # ---- plan input DMAs (x tiles), interleaved before first use ----
        xr = [tens[f"xr{k}"] for k in range(KT)]
        xs = [tens[f"xs{k}"] for k in range(KT)]
        gr = [tens[f"gr{k}"] for k in range(KT)]
        gs = [tens[f"gs{k}"] for k in range(KT)]

        def make_x_load(xlist, dram_name):
            pref = "xr" if dram_name == "xrT" else "xs"
            def x_load(k):
                sem = f"x{pref}{k}"
                def fn(e, _k=k, _nm=dram_name, _p=pref):
                    return e.dma_start(out=tens[f"{_p}{_k}"][:],
                                       in_=tens[_nm][_k * P:(_k + 1) * P, :])
                plan.op("sync", fn, incs=((sem, 16),))
                return sem
            st["x_load"][id(xlist)] = x_load

        make_x_load(xr, "xrT")
        make_x_load(xs, "xsT")

        plan_mlp(plan, st, C, "w13", "w2", xr, gr, "yrT")
        plan_mlp(plan, st, S, "w13s", "w2s", xs, gs, "ysT")

        # final completion: ACT waits for all output DMAs (per ring sem)
        for r in range(NSEM_OD):
            if plan.cnt.get(f"od{r}", 0):
                plan.wait("scalar", f"od{r}", plan.cnt[f"od{r}"])

        # ---- emit ----
        with ExitStack() as sem_ctx:
            sems = {}
            for name in plan.cnt:
                sems[name] = sem_ctx.enter_context(nc.semaphore(f"sem_{name}"))
            # sems that are only waited with value 0 don't appear; ensured by cnt

            with nc.Block() as block:
                def runner(stream):
                    def run(e):
                        for item in stream:
                            if item[0] == "wait":
                                _, s, v = item
                                e.wait_ge(sems[s], v)
                            else:
                                _, fn, incs = item
                                inst = fn(e)
                                rest = list(incs)
                                if rest and inst is not None:
                                    s, v = rest.pop(0)
                                    inst.then_inc(sems[s], v)
                                for s, v in rest:
                                    e.sem_inc(sems[s], v)
                    return run

                block.sync(runner(plan.streams["sync"]))
                block.tensor(runner(plan.streams["tensor"]))
                block.scalar(runner(plan.streams["scalar"]))
                block.vector(runner(plan.streams["vector"]))
    return nc


def _interleave_w13(w1e, w3e):
    d = w1e.shape[0]
    out = np.empty((d, 2 * HIDDEN), dtype=w1e.dtype)
    for m in range(HIDDEN // P):
        out[:, (2 * m) * P:(2 * m + 1) * P] = w1e[:, m * P:(m + 1) * P]
        out[:, (2 * m + 1) * P:(2 * m + 2) * P] = w3e[:, m * P:(m + 1) * P]
    return out


def route(xt, gate_w):
    logits = (xt @ gate_w.T).astype(np.float32)
    m = logits.max(axis=1, keepdims=True)
    e = np.exp(logits - m)
    scores = (e / e.sum(axis=1, keepdims=True)).astype(np.float32)
    sel = np.argsort(-scores, axis=1, kind="stable")[:, :TOP_K].astype(np.int32)
    top_scores = np.take_along_axis(scores, sel, axis=1)
    sel_flat = sel.reshape(-1)
    order = np.argsort(sel_flat, kind="stable")
    token_idx = (order // TOP_K).astype(np.int64)
    eid = sel_flat[order]
    scores_sorted = top_scores.reshape(-1)[order]
    return token_idx, eid, scores_sorted


def kernel(x, gate_w, w1, w2, w3, w1s, w2s, w3s, _run=None):
    x = np.asarray(x, dtype=np.float32)
    bs, slen, dim = x.shape
    N = bs * slen
    xt = np.ascontiguousarray(x.reshape(N, dim))
    S = N // N_CORES

    token_idx, eid, scores_sorted = route(xt, np.asarray(gate_w, np.float32))

    counts = np.bincount(eid, minlength=NUM_EXPERTS)
    C = int(max(256, ((counts.max() + 63) // 64) * 64))

    np_dt = mybir.dt.np(MM_DT)
    bounds = np.concatenate([[0], np.cumsum(counts)])
    w13s_i = _interleave_w13(np.asarray(w1s[0], np.float32),
                             np.asarray(w3s[0], np.float32)).astype(np_dt)
    w2s_c = np.ascontiguousarray(np.asarray(w2s[0], np.float32)).astype(np_dt)

    in_maps = []
    tok_per_core = []
    for e2 in range(N_CORES):
        lo, hi = int(bounds[e2]), int(bounds[e2 + 1])
        toks = token_idx[lo:hi]
        tok_per_core.append(toks)
        xr = np.zeros((C, dim), np.float32)
        xr[: hi - lo] = xt[toks] * scores_sorted[lo:hi, None]
        in_maps.append({
            "xrT": np.ascontiguousarray(xr.T).astype(np_dt),
            "xsT": np.ascontiguousarray(xt[e2 * S:(e2 + 1) * S].T).astype(np_dt),
            "w13": _interleave_w13(np.asarray(w1[e2], np.float32),
                                   np.asarray(w3[e2], np.float32)).astype(np_dt),
            "w2": np.ascontiguousarray(np.asarray(w2[e2], np.float32)).astype(np_dt),
            "w13s": w13s_i,
            "w2s": w2s_c,
        })

    nc = build_program(C, S, MM_DT)
    if _run is None:
        from concourse.bass_utils import run_bass_kernel_spmd
        results = run_bass_kernel_spmd(nc, in_maps, list(range(N_CORES))).results
    else:
        results = _run(nc, in_maps)

    out = np.empty((N, dim), np.float32)
    for e2 in range(N_CORES):
        out[e2 * S:(e2 + 1) * S] = results[e2]["ysT"].T
    for e2 in range(N_CORES):
        cnt = len(tok_per_core[e2])
        out[tok_per_core[e2]] += results[e2]["yrT"][:, :cnt].T
    return out.reshape(bs, slen, dim)

